# revision 1
# baseline (speedup 1.0000x reference)
"""KAN layer kernel for Trainium2 (8 NeuronCores, batch data-parallel).

Math: out = selu(x @ Wb + bias + einsum('bid,ijd,ij->bj', [1,t,t^2,t^3], spline, gate))
with t = tanh(x).  The einsum decomposes into 4 matmuls with W_d = spline[:,:,d]*gate;
the d=0 term is batch-independent and folds into the bias.  Per core (512 rows):
transpose x on PE (two 128x128 transposes into one (128,256) PSUM tile per
row-tile), tanh/square/cube once per row-tile in transposed layout, then
accumulate the 4 branch matmuls + a K=1 bias matmul in PSUM and apply selu as
lambda*relu(z) + lambda*alpha*exp(min(z,0)) - lambda*alpha
(tanh/exp/relu all live in the single 'exp_and_others' ACT table set).

Schedule notes:
- weights travel as bf16 (halves the dominant DMA traffic; matmuls run at
  1 cyc/row like fp32r).  Branch operands are bf16 too.  USE_BF16_WEIGHTS
  flips back to fp32r if more precision is ever needed.
- weight DRAM order is [w1, w2, w3, wb] and loads as two DMAs interleaved
  with the two x DMAs; matmuls run branch-major over {w1, w2} while later
  weights stream in, then per-row-tile tail groups {w3, wb, bias} finish
  each PSUM tile early so the selu chain and the two output stores overlap
  the remaining matmuls.
- bias rides the otherwise-idle SWDGE (Pool) path, off the HWDGE queue.
"""

import numpy as np
from contextlib import ExitStack

B, D, U = 4096, 256, 256
N_CORES = 8
BL = B // N_CORES          # 512 rows per core
NBT = BL // 128            # 4 output row-tiles per core
NKC = D // 128             # 2 contraction chunks

SELU_SCALE = 1.0507009873554805
SELU_ALPHA = 1.6732632423543772
LN_LA = float(np.log(np.float64(SELU_SCALE) * np.float64(SELU_ALPHA)))
NEG_LA = float(-(np.float64(SELU_SCALE) * np.float64(SELU_ALPHA)))

USE_BF16_WEIGHTS = True
PE_WARMUP_OPS = 11

TRACE = False
LAST_EXEC_NS = None
LAST_RESULTS = None

_compiled_nc = None


def _build():
    global _compiled_nc
    if _compiled_nc is not None:
        return _compiled_nc

    import concourse.bass as bass
    import concourse.mybir as mybir
    import concourse.tile as tile
    from concourse import bacc
    from concourse.masks import make_identity

    f32 = mybir.dt.float32
    f32r = mybir.dt.float32r
    wdt = mybir.dt.bfloat16 if USE_BF16_WEIGHTS else f32r
    Act = mybir.ActivationFunctionType
    Alu = mybir.AluOpType

    nc = bacc.Bacc("TRN2", target_bir_lowering=False, debug=False,
                   num_devices=N_CORES)

    x_d = nc.dram_tensor("x", [BL, D], f32, kind="ExternalInput").ap()
    # host packs weights in branch order [w1, w2, w3, wb]
    w_d = nc.dram_tensor("w", [4, D, U], wdt, kind="ExternalInput").ap()
    b_d = nc.dram_tensor("b", [1, U], f32r, kind="ExternalInput").ap()
    o_d = nc.dram_tensor("o", [BL, U], f32, kind="ExternalOutput").ap()

    # DRAM views with 128-row partition tiles in the free dims.
    x_v = x_d.rearrange("(g p) i -> g p i", p=128)                 # (4,128,256)
    w_v = w_d.rearrange("(a f) (c p) n -> a p f c n", a=2, p=128)  # (2,128,2,2,256)
    o_vg = o_d.rearrange("(g p) n -> g p n", p=128)                # (4,128,256)
    o_v = [o_d.rearrange("(h g p) n -> h p g n", h=2, p=128)[0]]   # (128,2,256)
    o_v2 = o_vg[2]
    o_v3 = o_vg[3]

    with tile.TileContext(nc) as tc, ExitStack() as ctx:
        consts = ctx.enter_context(tc.tile_pool(name="consts", bufs=1))
        xp = ctx.enter_context(tc.tile_pool(name="xp", bufs=2))
        tp = ctx.enter_context(tc.tile_pool(name="tp", bufs=4))
        op = ctx.enter_context(tc.tile_pool(name="op", bufs=4))
        pst = ctx.enter_context(
            tc.tile_pool(name="pst", bufs=3, space=bass.MemorySpace.PSUM))
        pso = ctx.enter_context(
            tc.tile_pool(name="pso", bufs=4, space=bass.MemorySpace.PSUM))

        # ---- input DMAs; program order = HWDGE queue order ----
        x01 = xp.tile([128, 2, 256], f32, tag="x01")
        nc.sync.dma_start(out=x01[:], in_=x_v[0:2].rearrange("g p i -> p g i"))
        wA = consts.tile([128, 2, 2, 256], wdt, tag="wA")   # branches 0(base),1
        nc.sync.dma_start(out=wA[:], in_=w_v[0])
        x23 = xp.tile([128, 2, 256], f32, tag="x23")
        nc.sync.dma_start(out=x23[:], in_=x_v[2:4].rearrange("g p i -> p g i"))
        wB = consts.tile([128, 2, 2, 256], wdt, tag="wB")   # branches 2,3
        nc.sync.dma_start(out=wB[:], in_=w_v[1])
        x_slice = [x01[:, 0], x01[:, 1], x23[:, 0], x23[:, 1]]
        # w_br[br] -> (128, 2, 256) view
        w_br = {0: wA[:, 0], 1: wA[:, 1], 2: wB[:, 0], 3: wB[:, 1]}

        # constants: identity for PE transpose, ones row + bias for the K=1
        # bias matmul, exp-bias column.
        ident = consts.tile([128, 128], f32, tag="ident")
        make_identity(nc, ident)
        ones_f = consts.tile([1, 128], f32, tag="ones_f")
        nc.vector.memset(ones_f, 1.0)
        ones_r = consts.tile([1, 128], f32r, tag="ones")
        nc.vector.tensor_copy(ones_r[:], ones_f[:])
        bias_sb = consts.tile([1, U], f32r, tag="bias")
        nc.gpsimd.dma_start(out=bias_sb[:], in_=b_d)
        lnla_sb = consts.tile([128, 1], f32, tag="lnla")
        nc.vector.memset(lnla_sb, LN_LA)
        # dependency-free activation: forces the exp_and_others ACT table
        # load to happen during the input DMAs, not on the tanh critical path
        warm = consts.tile([1, 1], f32, tag="warm")
        nc.scalar.activation(warm[:], ones_f[:, :1], Act.Exp)
        # PE warmup: dependency-free transposes keep the PE busy through the
        # input-DMA wait so the p-state ramp reaches full speed before the
        # real transposes/matmuls arrive.  warm_src is DVE-memset (ready
        # ~1us, much earlier than make_identity's Pool path).
        warm_src = consts.tile([128, 128], f32, tag="warm_src")
        nc.vector.memset(warm_src, 0.0)
        scr = pst.tile([128, 128], f32, tag="scr", bufs=1)
        for _ in range(PE_WARMUP_OPS):
            nc.tensor.transpose(scr[:], warm_src[:], warm_src[:])

        # ---- pipelined body ----
        branches = [[None] * NBT for _ in range(4)]  # [br][bt] -> (128,256)
        po = [pso.tile([128, U], f32, tag="po", name=f"po{bt}")
              for bt in range(NBT)]
        res_h0 = op.tile([128, 2, 256], f32, tag="res01", bufs=1)
        res_2 = op.tile([128, 256], f32, tag="res2", bufs=1)
        res_3 = op.tile([128, 256], f32, tag="res3", bufs=1)
        res_slice = [res_h0[:, 0, :], res_h0[:, 1, :], res_2[:], res_3[:]]

        def powers(bt):
            """PE-transpose x row-tile bt, then tanh/square/cube (bf16)."""
            xt = x_slice[bt]
            ps = pst.tile([128, 256], f32, tag="tp", name=f"ps{bt}")
            for kc in range(NKC):
                nc.tensor.transpose(ps[:, kc * 128:(kc + 1) * 128],
                                    xt[:, kc * 128:(kc + 1) * 128], ident[:])
            xT = tp.tile([128, 256], wdt, tag="xT", name=f"xT{bt}")
            nc.vector.tensor_copy(xT[:], ps[:])
            t1 = tp.tile([128, 256], wdt, tag="t1", name=f"t1_{bt}")
            nc.scalar.activation(t1[:], ps[:], Act.Tanh)
            t2 = tp.tile([128, 256], wdt, tag="t2", name=f"t2_{bt}")
            nc.vector.tensor_mul(t2[:], t1[:], t1[:])
            t3 = tp.tile([128, 256], wdt, tag="t3", name=f"t3_{bt}")
            nc.vector.tensor_mul(t3[:], t2[:], t1[:])
            branches[0][bt] = xT
            branches[1][bt] = t1
            branches[2][bt] = t2
            branches[3][bt] = t3

        def early_mms(bt):
            """branches 0 (xT) and 1 (t); their weights arrive first."""
            for br in (0, 1):
                for kc in range(NKC):
                    nc.tensor.matmul(
                        po[bt][:],
                        branches[br][bt][:, kc * 128:(kc + 1) * 128],
                        w_br[br][:, kc, :],
                        start=(br == 0 and kc == 0), stop=False)

        def tail(bt):
            """finish po[bt] with {w2, w3, bias}, then selu into res."""
            for br in (2, 3):
                for kc in range(NKC):
                    nc.tensor.matmul(
                        po[bt][:],
                        branches[br][bt][:, kc * 128:(kc + 1) * 128],
                        w_br[br][:, kc, :], start=False, stop=False)
            nc.tensor.matmul(po[bt][:], ones_r[:], bias_sb[:],
                             start=False, stop=True)
            # selu(z) = scale*relu(z) + scale*alpha*exp(min(z,0)) - scale*alpha
            neg = op.tile([128, U], f32, tag="neg", name=f"neg{bt}")
            nc.vector.tensor_scalar_min(neg[:], po[bt][:], 0.0)
            e3 = op.tile([128, U], f32, tag="e3", name=f"e3_{bt}")
            nc.scalar.activation(e3[:], neg[:], Act.Exp, bias=lnla_sb[:])
            pos = op.tile([128, U], f32, tag="pos", name=f"pos{bt}")
            if bt < 4:
                nc.scalar.activation(pos[:], po[bt][:], Act.Relu,
                                     scale=SELU_SCALE)
            else:
                # tail row-tiles: relu on DVE to shorten the serial ACT tail
                nc.vector.tensor_scalar(pos[:], po[bt][:], SELU_SCALE, 0.0,
                                        Alu.mult, Alu.max)
            nc.vector.scalar_tensor_tensor(res_slice[bt], e3[:], NEG_LA,
                                           pos[:], Alu.add, Alu.add)

        powers(0)
        powers(1)
        early_mms(0)
        early_mms(1)
        powers(2)
        powers(3)
        tail(0)
        tail(1)
        nc.sync.dma_start(out=o_v[0], in_=res_h0[:])
        early_mms(2)
        early_mms(3)
        tail(2)
        nc.sync.dma_start(out=o_v2, in_=res_2[:])
        tail(3)
        nc.sync.dma_start(out=o_v3, in_=res_3[:])

    nc.compile()
    _compiled_nc = nc
    return nc


def kernel(**inputs):
    global LAST_EXEC_NS, LAST_RESULTS
    import ml_dtypes

    x = np.ascontiguousarray(inputs["inputs"], dtype=np.float32)
    bw = np.asarray(inputs["base_weight"], dtype=np.float32)
    bias = np.asarray(inputs["bias"], dtype=np.float32)
    sw = np.asarray(inputs["spline_weights"], dtype=np.float32)
    gw = np.asarray(inputs["gate_weights"], dtype=np.float32)

    # branch order [base, w1, w2, w3] to match the kernel's DMA grouping
    wall = np.empty((4, D, U), np.float32)
    wall[0] = bw
    for d in (1, 2, 3):
        wall[d] = sw[:, :, d] * gw
    if USE_BF16_WEIGHTS:
        wall = wall.astype(ml_dtypes.bfloat16)
    bias_total = (bias + (sw[:, :, 0] * gw).sum(axis=0)).astype(
        np.float32).reshape(1, U)

    nc = _build()
    from concourse.bass_utils import run_bass_kernel_spmd

    in_maps = [
        {"x": np.ascontiguousarray(x[i * BL:(i + 1) * BL]),
         "w": wall, "b": bias_total}
        for i in range(N_CORES)
    ]
    res = run_bass_kernel_spmd(nc, in_maps, core_ids=list(range(N_CORES)),
                               trace=TRACE)
    LAST_EXEC_NS = res.exec_time_ns
    LAST_RESULTS = res
    return np.concatenate([r["o"] for r in res.results], axis=0)

